# revision 64
# baseline (speedup 1.0000x reference)
"""Block-sparse causal self-attention on 8 TRN2 NeuronCores (SPMD Bass/Tile kernel).

Sharding: core c -> (batch b = c//2, head-group g = c%2 of 6 heads).
Each core computes qkv projection (its 6 heads), masked attention, and a
partial output projection (its 384 rows of W_proj).  Host sums the two
partials per batch and concatenates batches.

Token reorder (host-side permutation, inverted on output):
  [A | U_0 .. U_7]  with U_i = [tactile_i (16), image_i (196)], A = 9 actions.
Actions-first makes every query chunk's visible key set a clean prefix of
128-wide key tiles; partial visibility inside a tile is handled by an
elementwise multiply restricted to the bounding box of the masked region,
and by restricting compute to the visible (row, column) window.

All matmul inputs are float16 (f32 PSUM accumulation).  Attention is
computed in transposed layout S^T[k, q]; softmax normalization comes from
ones-columns appended to V (rowsum lands in the PV matmul output) and a
small f32r broadcast-matmul computes 1/rowsum per query.

Emission is interleaved so the PE never starves: qkv/V/output-projection
chains are woven between attention items as fillers, attention runs a
1-deep software pipeline (S(i+1) issued before PV(i)), and DMAs are
ordered so the first projection chain starts ~1-2us into the kernel.
"""

import os
import sys
from contextlib import ExitStack

import numpy as np

for _p in ("/opt/trn_rl_repo", "/root/.axon_site/_ro/trn_rl_repo"):
    if os.path.isdir(_p) and _p not in sys.path:
        sys.path.insert(0, _p)

import concourse.bass as bass
import concourse.tile as tile
from concourse import mybir
from concourse.bass_utils import run_bass_kernel_spmd

F32 = mybir.dt.float32
F32R = mybir.dt.float32r
F16 = mybir.dt.float16
AF = mybir.ActivationFunctionType

L, PP, PT = 8, 196, 16
T, C, NH, B, HD = 1705, 768, 12, 4, 64
NCORES = 8
NHG = NH // 2          # heads per core = 6
NPACK = NHG // 2       # head pairs per core = 3
KC = C // 128          # 6 contraction tiles over C
KT = 128               # key tile size
NKT = (T + KT - 1) // KT   # 14
TP = 1706              # T padded to even
QCH = [(0, 424), (424, 848), (848, 1272), (1272, T)]
QCHC = [(0, 424), (424, 848), (848, 1272), (1272, TP)]  # compute chunks (even n)
HD1 = 2 * HD           # V width: 64 V cols | 64 ones cols (rowsum lands
                       # replicated in PV-output partitions 64..127)
VW = NHG * HD1         # 768


def _perm():
    idx = list(range(0, 9))          # actions first
    for i in range(L):
        idx += list(range(9 + PT * i, 9 + PT * (i + 1)))
        idx += list(range(9 + L * PT + PP * i, 9 + L * PT + PP * (i + 1)))
    return np.asarray(idx, dtype=np.int64)


PERM = _perm()


def _analyze(mask_perm):
    """Compile-time plan from the (permuted) boolean mask.

    Returns (plan, mpack):
      plan: per query-chunk, tuple of items
            (kt, kw, kwv, c0, bbox-or-None, moff) where
              kwv  = even-padded count of visible key rows computed,
              c0   = first (even-aligned) visible query column,
              bbox = (br0, br1, bc0, bc1) of the masked (zero) region inside
                     the computed window [kwv rows x (n-c0) cols]; bc* are
                     relative to c0; moff = column offset into mpack.
      mpack: [128, Wtot] float packed mask bounding boxes (S^T layout).
    """
    plan = []
    cols = []
    widths = 0
    cioff = []
    for (q0, q1) in QCH:
        ci_start = widths
        sub = mask_perm[q0:q1, :]
        nq = q1 - q0
        items = []
        for kt in range(NKT):
            k0, k1 = kt * KT, min((kt + 1) * KT, T)
            mt = sub[:, k0:k1].T          # [kw, nq]  S^T layout
            if not mt.any():
                continue
            kw = k1 - k0
            cvis = np.nonzero(mt.any(axis=0))[0]
            c0 = int(cvis[0]) & ~1
            c1 = int(cvis[-1]) + 1
            c1 += c1 & 1
            # c1m == -1 means "extends to the (padded) chunk end"
            c1m = -1 if c1 >= nq else c1
            rvis = np.nonzero(mt.any(axis=1))[0]
            r1 = int(rvis[-1]) + 1
            kwv = r1 + (r1 & 1)
            # computed window: rows [0:kwv], cols [c0:c1]; rows >= kw (tile
            # pad) get False here but their V rows are zeroed, so either way
            # they contribute nothing.
            cw = min(c1, nq)
            win = np.zeros((kwv, cw - c0), dtype=bool)
            rlim = min(r1, kw)
            win[0:rlim, :] = mt[0:rlim, c0:cw]
            if kwv > kw:
                win[kw:kwv, :] = True     # pad rows: harmless (V rows zero)
            z = ~win
            if z.any():
                rr = np.nonzero(z.any(axis=1))[0]
                cc = np.nonzero(z.any(axis=0))[0]
                br0, br1 = int(rr[0]), int(rr[-1]) + 1
                br0 = 0 if br0 < 64 else 64      # vector-engine partition window
                bc0, bc1 = int(cc[0]), int(cc[-1]) + 1
                frag = np.ones((128, bc1 - bc0), np.float32)
                frag[br0:br1, :] = win[br0:br1, bc0:bc1].astype(np.float32)
                items.append((kt, kw, kwv, c0, c1m, (br0, br1, bc0, bc1), widths))
                cols.append(frag)
                widths += bc1 - bc0
            else:
                items.append((kt, kw, kwv, c0, c1m, None, 0))
        # reorder: first item must cover [0:n] (start) and last item must
        # cover [0:n] (stop); move the second full-width item to the end.
        full = [i for i, it in enumerate(items) if it[3] == 0 and it[4] == -1]
        assert len(full) >= 2 and full[0] == 0, "need >=2 full-width items"
        j = full[1]
        items = [items[0]] + items[1:j] + items[j + 1:] + [items[j]]
        plan.append(tuple(items))
        cioff.append((ci_start, widths))
    if widths == 0:
        mpack = np.zeros((128, 4), np.float32)
    else:
        mpack = np.concatenate(cols, axis=1)
    return tuple(plan), np.ascontiguousarray(mpack), tuple(cioff)


_BUILD_CACHE = {}


def _split_excess_waits(nc, max_waits=1):
    """walrus (this build) rejects instructions with >2 sem-wait commands.

    Tile's kernel-tail drain waits on every live semaphore in one Drain;
    split the excess onto preceding same-engine instructions (extra Drains
    for InstDrain, NoOps otherwise).
    """
    import copy

    for bb in nc.main_func.blocks:
        insts = bb.instructions
        i = 0
        while i < len(insts):
            ins = insts[i]
            si = ins.sync_info
            mw = max_waits
            if si is not None and len(si.on_wait) > mw:
                waits = list(si.on_wait)
                extra = waits[:-mw]
                newones = []
                for j in range(0, len(extra), max_waits):
                    if ins.__class__.__name__ == "InstDrain":
                        d = mybir.InstDrain(
                            name=f"{ins.name}-sw{j}", engine=ins.engine
                        )
                    else:
                        d = mybir.InstNoOp(name=f"{ins.name}-sw{j}", engine=ins.engine)
                    si2 = copy.deepcopy(si)
                    si2.on_wait = extra[j:j + max_waits]
                    si2.on_update = []
                    d.sync_info = si2
                    newones.append(d)
                si.on_wait = waits[-mw:]
                for d in reversed(newones):
                    insts.insert(i, d)
                i += len(newones)
            i += 1


def _build(plan, wtot, cioff, split=True):
    key = (tuple(plan), wtot, cioff, split)
    if key in _BUILD_CACHE:
        return _BUILD_CACHE[key]

    nc = bass.Bass()
    xT = nc.declare_dram_parameter("xT", [C, TP], F16, isOutput=False)
    wa = nc.declare_dram_parameter("wa", [C, 3 * NHG * HD], F16, isOutput=False)
    wp = nc.declare_dram_parameter("wp", [NHG * HD, C], F16, isOutput=False)
    mp = nc.declare_dram_parameter("mp", [128, max(wtot, 4)], F16, isOutput=False)
    out = nc.declare_dram_parameter("out", [T, C], F32, isOutput=True)

    with tile.TileContext(nc) as tc:
        with ExitStack() as ctx:
            const = ctx.enter_context(tc.tile_pool(name="const", bufs=1))

            # ---- SBUF-resident tensors -------------------------------
            xt_sb = [const.tile([128, TP], F16, tag=f"xt{k}", name=f"xt{k}")
                     for k in range(KC)]
            wa_sb = [const.tile([128, 3 * NHG * HD], F16, tag=f"wa{k}", name=f"wa{k}")
                     for k in range(KC)]
            wp_sb = [const.tile([128, C], F16, tag=f"wp{k}", name=f"wpt{k}")
                     for k in range(3)]
            qt_sb = [const.tile([128, TP], F16, tag=f"qt{p}", name=f"qt{p}")
                     for p in range(NPACK)]
            kt_sb = [const.tile([128, TP], F16, tag=f"kt{p}", name=f"ktt{p}")
                     for p in range(NPACK)]
            v6_sb = [const.tile([128, VW], F16, tag=f"v6{t}", name=f"v6{t}")
                     for t in range(NKT)]
            yt_sb = [const.tile([128, TP], F16, tag=f"yt{p}", name=f"yt{p}")
                     for p in range(NPACK)]
            mkall = const.tile([128, max(wtot, 4)], F16, tag="mkall", name="mkall")


            # ---- DMA issue order -------------------------------------
            # x/wa interleaved per contraction tile (first chain starts
            # ~1us in); mask-frag ranges woven in, ordered by the chunk
            # processing order [1, 2, 3, 0]; wp last (needed only by the
            # output projection, which starts much later).
            def dma_mk(ci):
                m0, m1 = cioff[ci]
                if m1 > m0:
                    nc.sync.dma_start(out=mkall[:, m0:m1], in_=mp[:, m0:m1])

            for k in range(KC):
                nc.sync.dma_start(out=xt_sb[k][:, :], in_=xT[k * 128:(k + 1) * 128, :])
                nc.sync.dma_start(out=wa_sb[k][:, :], in_=wa[k * 128:(k + 1) * 128, :])
            for ci in (1, 2, 3, 0):
                dma_mk(ci)
            for k in range(3):
                nc.sync.dma_start(out=wp_sb[k][:, :], in_=wp[k * 128:(k + 1) * 128, :])

            # ---- emission helpers ------------------------------------
            def qkv_chain(pool, p, ci, j):
                """Q^T (j=0) or K^T (j=1) chain for pack p, chunk ci."""
                q0, q1 = QCHC[ci]
                n = q1 - q0
                dst = (qt_sb, kt_sb)[j][p]
                ps = pool.tile([128, 512], F32, tag="pj", name="pjt")
                col = j * NHG * HD + p * 128
                for k in range(KC):
                    nc.tensor.matmul(
                        ps[:, 0:n],
                        wa_sb[k][:, col:col + 128],
                        xt_sb[k][:, q0:q1],
                        start=(k == 0), stop=(k == KC - 1),
                    )
                nc.vector.tensor_copy(dst[:, q0:q1], ps[:, 0:n])

            def v_chain(pool, t):
                tw = min(128, T - t * KT)
                twp = tw + (tw & 1)
                ps = pool.tile([128, 512], F32, tag="pj", name="pjt")
                for k in range(KC):
                    nc.tensor.matmul(
                        ps[0:twp, 0:NHG * HD],
                        xt_sb[k][:, t * KT:t * KT + twp],
                        wa_sb[k][:, 2 * NHG * HD:3 * NHG * HD],
                        start=(k == 0), stop=(k == KC - 1),
                    )
                v6v = v6_sb[t].rearrange("a (h d) -> a h d", d=HD1)
                if tw < 128:
                    nc.gpsimd.memset(v6_sb[t][:, :], 0.0)
                psv = ps[:, 0:NHG * HD].rearrange("a (h d) -> a h d", d=HD)
                nc.vector.tensor_copy(v6v[0:tw, :, 0:HD], psv[0:tw, :, :])
                nc.gpsimd.memset(v6v[0:tw, :, HD:HD1], 1.0)

            def d_chain(pool, osb, t, half, wide=False):
                tw = min(128, T - t * KT)
                twp = tw + (tw & 1)
                n0, n1 = (0, 384) if half == 0 else (384, 768)
                if wide:
                    # borrow an idle sps slot, using its native tag/shape
                    po = pool.tile([128, 2, 512], F32, tag="s", name="st")[:, 0, :]
                else:
                    po = pool.tile([128, 512], F32, tag="pj", name="pjt")
                for k3 in range(3):
                    nc.tensor.matmul(
                        po[0:twp, 0:384],
                        yt_sb[k3][:, t * KT:t * KT + twp],
                        wp_sb[k3][:, n0:n1],
                        start=(k3 == 0), stop=(k3 == 2),
                    )
                ot = osb.tile([128, 384], F32, tag="ot", name="ot_sb")
                nc.vector.tensor_copy(ot[0:tw, :], po[0:tw, :384])
                # alternate store queues: serial issue on one queue (~0.65us
                # per DMA) otherwise dominates the kernel-end drain
                eng = nc.sync if half == 0 else nc.gpsimd
                eng.dma_start(
                    out=out[t * KT:t * KT + tw, n0:n1], in_=ot[0:tw, :]
                )

            # ---- stage 1: chains needed before attention -------------
            # chunks are processed in order [1, 2, 3, 0]: chunk 0 holds the
            # action queries (which see every key tile), so it runs last,
            # after all kt/v6 tiles exist.  C(1) needs kt tiles 0..6 (in
            # key-chunks 0..2) and v6 tiles 0..6.
            #
            # Wave A interleaves 8 accumulation chains c-tile-major so the
            # PE always has a ready matmul while the x/wa DMAs stream in
            # (chain-major order would block in-order PE issue on the last
            # c-tile's DMA).  Wave B runs after the DMAs have landed.
            def chain_mm(ps, spec, k, start, stop):
                kind, a, b_ = spec
                if kind == "v":
                    t = a
                    tw = min(128, T - t * KT)
                    twp = tw + (tw & 1)
                    nc.tensor.matmul(
                        ps[0:twp, 0:NHG * HD],
                        xt_sb[k][:, t * KT:t * KT + twp],
                        wa_sb[k][:, 2 * NHG * HD:3 * NHG * HD],
                        start=start, stop=stop,
                    )
                else:
                    p, ci, j = a, b_, 0 if kind == "q" else 1
                    q0, q1 = QCHC[ci]
                    col = j * NHG * HD + p * 128
                    nc.tensor.matmul(
                        ps[:, 0:q1 - q0],
                        wa_sb[k][:, col:col + 128],
                        xt_sb[k][:, q0:q1],
                        start=start, stop=stop,
                    )

            def chain_fin(ps, spec):
                kind, a, b_ = spec
                if kind == "v":
                    t = a
                    tw = min(128, T - t * KT)
                    v6v = v6_sb[t].rearrange("a (h d) -> a h d", d=HD1)
                    if tw < 128:
                        # rows >= tw must be zero (pad keys contribute 0)
                        nc.gpsimd.memset(v6_sb[t][:, :], 0.0)
                    psv = ps[:, 0:NHG * HD].rearrange("a (h d) -> a h d", d=HD)
                    nc.vector.tensor_copy(v6v[0:tw, :, 0:HD], psv[0:tw, :, :])
                    nc.gpsimd.memset(v6v[0:tw, :, HD:HD1], 1.0)
                else:
                    p, ci = a, b_
                    q0, q1 = QCHC[ci]
                    dst = (qt_sb, kt_sb)[0 if kind == "q" else 1][p]
                    nc.vector.tensor_copy(dst[:, q0:q1], ps[:, 0:q1 - q0])

            # minimal prefix: exactly what C(1) pack 0's S matmuls need up
            # front (4 chains, c-tile-major so the PE tracks the DMA
            # stream).  Their DVE copies then overlap the v6 0..3 chains,
            # so C(1)'s first item starts ~2us earlier than with all 8
            # chains in one wave (whose 8 copies all queue at DMA-end).
            # Other packs' chains and v6 4..6 are fillers inside C(1).
            waveA = [("k", 0, 0), ("k", 0, 1), ("k", 0, 2), ("q", 0, 1)]
            with tc.tile_pool(name="pb", bufs=8, space="PSUM") as pb:
                psA = [pb.tile([128, 512], F32, tag="pj", name="pjt")
                       for _ in waveA]
                for k in range(KC):
                    for ps, spec in zip(psA, waveA):
                        chain_mm(ps, spec, k, k == 0, k == KC - 1)
                for ps, spec in zip(psA, waveA):
                    chain_fin(ps, spec)
                for t in range(4):
                    ps = pb.tile([128, 512], F32, tag="pj", name="pjt")
                    for k in range(KC):
                        chain_mm(ps, ("v", t, None), k, k == 0, k == KC - 1)
                    chain_fin(ps, ("v", t, None))

            # ---- main interleaved phase ------------------------------
            with tc.tile_pool(name="sps", bufs=2, space="PSUM") as sps, \
                 tc.tile_pool(name="ups", bufs=2, space="PSUM") as ups, \
                 tc.tile_pool(name="proj", bufs=2, space="PSUM") as proj, \
                 tc.tile_pool(name="epool", bufs=4) as epool, \
                 tc.tile_pool(name="npool", bufs=6) as npool, \
                 tc.tile_pool(name="osb", bufs=2) as osb:

                def emit_pv(p, u2, et, kt, kwv, c0, c1e, first, last):
                    for e in (0, 1):
                        h = 2 * p + e
                        nc.tensor.matmul(
                            u2[e][0:HD1, c0:c1e],
                            v6_sb[kt][0:kwv, h * HD1:(h + 1) * HD1],
                            et[0:kwv, e, c0:c1e],
                            start=first, stop=last,
                        )

                # filler emitters per chunk index, consumed between items.
                # chunk processing order is [1, 2, 3, 0]; a filler emitted
                # during chunk ci may only depend on chunks processed before.
                def fillers_for(ci):
                    fs = []
                    if ci == 1:
                        # v6 4..6 first: C(1) pack 0's own items read them a
                        # few slots after the weave emits them.
                        for t in (4, 5, 6):
                            fs.append(lambda t=t: v_chain(proj, t))
                        for p in (1, 2):         # pack p's qt/kt for C(1)
                            fs.append(lambda p=p: qkv_chain(proj, p, 1, 0))
                            fs.append(lambda p=p: qkv_chain(proj, p, 0, 1))
                            fs.append(lambda p=p: qkv_chain(proj, p, 1, 1))
                            fs.append(lambda p=p: qkv_chain(proj, p, 2, 1))
                        for t in (7, 8, 9, 10):  # C(2) needs v6 7..10
                            fs.append(lambda t=t: v_chain(proj, t))
                        for p in range(NPACK):   # C(2) needs kt ch3, qt ch2
                            fs.append(lambda p=p: qkv_chain(proj, p, 3, 1))
                        for p in range(NPACK):
                            fs.append(lambda p=p: qkv_chain(proj, p, 2, 0))
                    elif ci == 2:
                        for t in (11, 12, 13):
                            fs.append(lambda t=t: v_chain(proj, t))
                        for p in range(NPACK):
                            fs.append(lambda p=p: qkv_chain(proj, p, 3, 0))  # qt ch3
                        for t in (4, 5):         # rows in [512,768) < ch1 end
                            for h in (0, 1):
                                fs.append(lambda t=t, h=h: d_chain(proj, osb, t, h))
                    elif ci == 3:
                        for p in range(NPACK):
                            fs.append(lambda p=p: qkv_chain(proj, p, 0, 0))  # qt ch0
                        for t in (6, 7, 8):      # rows in [768,1152) < ch2 end
                            for h in (0, 1):
                                fs.append(lambda t=t, h=h: d_chain(proj, osb, t, h))
                    elif ci == 0:
                        for t in (9, 10, 11, 12, 13):   # rows >= 1152, ch2+ch3
                            for h in (0, 1):
                                fs.append(lambda t=t, h=h: d_chain(proj, osb, t, h))
                    return fs

                for ci in (1, 2, 3, 0):
                    q0, q1 = QCHC[ci]
                    n = q1 - q0
                    items = plan[ci]
                    fillers = fillers_for(ci)
                    fi = 0
                    # spread fillers evenly across this chunk's items
                    tot = len(items) * NPACK
                    done = 0
                    for p in range(NPACK):
                        u2 = [ups.tile([HD1, 448], F32, tag="u", name="ut")
                              for _ in (0, 1)]
                        pend = None   # deferred PV args
                        for ii, (kt, kw, kwv, c0, c1m, bbox, moff) in enumerate(items):
                            c1e = n if c1m < 0 else c1m
                            st = sps.tile([128, 2, 512], F32, tag="s", name="st")
                            et = epool.tile([128, 2, 512], F16, tag="e", name="et")
                            for e in (0, 1):
                                nc.tensor.matmul(
                                    st[0:kwv, e, c0:c1e],
                                    kt_sb[p][e * 64:(e + 1) * 64,
                                             kt * KT:kt * KT + kwv],
                                    qt_sb[p][e * 64:(e + 1) * 64,
                                             q0 + c0:q0 + c1e],
                                    start=True, stop=True,
                                )
                            nc.scalar.activation(
                                et[0:kwv, :, c0:c1e], st[0:kwv, :, c0:c1e],
                                AF.Exp, scale=0.125,
                            )
                            if bbox is not None:
                                br0, br1, bc0, bc1 = bbox
                                ets = et[br0:br1, :, c0 + bc0:c0 + bc1]
                                nc.vector.tensor_mul(
                                    ets,
                                    ets,
                                    mkall[br0:br1, moff:moff + bc1 - bc0]
                                    .rearrange("a (o w) -> a o w", o=1)
                                    .to_broadcast(ets.shape),
                                )
                            if pend is not None:
                                emit_pv(p, u2, *pend)
                            pend = (et, kt, kwv, c0, c1e, ii == 0, False)
                            done += 1
                            while fi * tot < done * len(fillers):
                                fillers[fi]()
                                fi += 1
                        et, kt, kwv, c0, c1e, first, _ = pend
                        emit_pv(p, u2, et, kt, kwv, c0, c1e, first, True)
                        # softmax normalization: PV already replicated the
                        # rowsum into u2 partitions 64..127 (ones-cols in V),
                        # so this is a pure-DVE reciprocal + multiply.
                        # 1/rowsum as Exp(-Ln(r)): stays inside the Exp/Ln
                        # activation table (a Reciprocal activation would
                        # trigger a ~1.3us ACT_TABLE_LOAD on every switch).
                        for e in (0, 1):
                            lr = npool.tile([64, 448], F32, tag="lr", name="lr")
                            nc.scalar.activation(
                                lr[:, 0:n], u2[e][64:128, 0:n], AF.Ln
                            )
                            rs = npool.tile([64, 448], F32, tag="rs", name="rs")
                            nc.scalar.activation(
                                rs[:, 0:n], lr[:, 0:n], AF.Exp, scale=-1.0
                            )
                            nc.vector.tensor_mul(
                                yt_sb[p][e * 64:(e + 1) * 64, q0:q1],
                                u2[e][0:64, 0:n],
                                rs[:, 0:n],
                            )
                    while fi < len(fillers):
                        fillers[fi]()
                        fi += 1

                # ---- tail: output tiles touching chunk-0 rows --------
                # borrow the now-idle sps slots so 4 chains pipeline
                for i, (t, h) in enumerate(
                        [(t, h) for t in (0, 1, 2, 3) for h in (0, 1)]):
                    if i % 2 == 0:
                        d_chain(proj, osb, t, h)
                    else:
                        d_chain(sps, osb, t, h, wide=True)

    if split:
        _split_excess_waits(nc)
    _BUILD_CACHE[key] = nc
    return nc


def _prep_inputs(x, W_attn, W_proj, mpack):
    """Per-core input maps. core c -> batch c//2, head-group c%2."""
    x = np.asarray(x, np.float32)
    W_attn = np.asarray(W_attn, np.float32)
    W_proj = np.asarray(W_proj, np.float32)
    mpack16 = mpack.astype(np.float16)
    in_maps = []
    xT_by_batch = []
    for b in range(B):
        xt = np.zeros((C, TP), np.float16)
        xt[:, :T] = x[b][PERM, :].T.astype(np.float16)
        xT_by_batch.append(xt)
    for c in range(NCORES):
        b, g = c // 2, c % 2
        cs = slice(g * NHG * HD, (g + 1) * NHG * HD)
        wa_s = np.ascontiguousarray(
            np.concatenate(
                [W_attn[:, cs], W_attn[:, C:][:, cs], W_attn[:, 2 * C:][:, cs]],
                axis=1,
            ).astype(np.float16)
        )
        wp_s = np.ascontiguousarray(W_proj[cs, :].astype(np.float16))
        in_maps.append(
            {"xT": xT_by_batch[b], "wa": wa_s, "wp": wp_s, "mp": mpack16}
        )
    return in_maps


def _run(inputs, trace=False, trace_cores=None):
    x = np.asarray(inputs["x"], np.float32)
    mask = np.asarray(inputs["mask"], bool)
    mask_perm = mask[np.ix_(PERM, PERM)]
    plan, mpack, cioff = _analyze(mask_perm)
    nc = _build(plan, mpack.shape[1], cioff)
    in_maps = _prep_inputs(x, inputs["W_attn"], inputs["W_proj"], mpack)
    res = run_bass_kernel_spmd(
        nc, in_maps, list(range(NCORES)), trace=trace, trace_cores=trace_cores
    )
    outs = [np.asarray(r["out"], np.float32) for r in res.results]
    full = np.empty((B, T, C), np.float32)
    for b in range(B):
        comb = outs[2 * b] + outs[2 * b + 1]
        full[b][PERM, :] = comb
    return full, res


def kernel(**inputs) -> np.ndarray:
    out, _ = _run(inputs)
    return out


# revision 66
# speedup vs baseline: 1.0143x; 1.0143x over previous
"""Block-sparse causal self-attention on 8 TRN2 NeuronCores (SPMD Bass/Tile kernel).

Sharding: core c -> (batch b = c//2, head-group g = c%2 of 6 heads).
Each core computes qkv projection (its 6 heads), masked attention, and a
partial output projection (its 384 rows of W_proj).  Host sums the two
partials per batch and concatenates batches.

Token reorder (host-side permutation, inverted on output):
  [A | U_0 .. U_7]  with U_i = [tactile_i (16), image_i (196)], A = 9 actions.
Actions-first makes every query chunk's visible key set a clean prefix of
128-wide key tiles; partial visibility inside a tile is handled by an
elementwise multiply restricted to the bounding box of the masked region,
and by restricting compute to the visible (row, column) window.

All matmul inputs are float16 (f32 PSUM accumulation).  Attention is
computed in transposed layout S^T[k, q]; softmax normalization comes from
ones-columns appended to V (rowsum lands in the PV matmul output) and a
small f32r broadcast-matmul computes 1/rowsum per query.

Emission is interleaved so the PE never starves: qkv/V/output-projection
chains are woven between attention items as fillers, attention runs a
1-deep software pipeline (S(i+1) issued before PV(i)), and DMAs are
ordered so the first projection chain starts ~1-2us into the kernel.
"""

import os
import sys
from contextlib import ExitStack

import numpy as np

for _p in ("/opt/trn_rl_repo", "/root/.axon_site/_ro/trn_rl_repo"):
    if os.path.isdir(_p) and _p not in sys.path:
        sys.path.insert(0, _p)

import concourse.bass as bass
import concourse.tile as tile
from concourse import mybir
from concourse.bass_utils import run_bass_kernel_spmd

F32 = mybir.dt.float32
F32R = mybir.dt.float32r
F16 = mybir.dt.float16
AF = mybir.ActivationFunctionType

L, PP, PT = 8, 196, 16
T, C, NH, B, HD = 1705, 768, 12, 4, 64
NCORES = 8
NHG = NH // 2          # heads per core = 6
NPACK = NHG // 2       # head pairs per core = 3
KC = C // 128          # 6 contraction tiles over C
KT = 128               # key tile size
NKT = (T + KT - 1) // KT   # 14
TP = 1706              # T padded to even
QCH = [(0, 424), (424, 848), (848, 1272), (1272, T)]
QCHC = [(0, 424), (424, 848), (848, 1272), (1272, TP)]  # compute chunks (even n)
HD1 = 2 * HD           # V width: 64 V cols | 64 ones cols (rowsum lands
                       # replicated in PV-output partitions 64..127)
VW = NHG * HD1         # 768


def _perm():
    idx = list(range(0, 9))          # actions first
    for i in range(L):
        idx += list(range(9 + PT * i, 9 + PT * (i + 1)))
        idx += list(range(9 + L * PT + PP * i, 9 + L * PT + PP * (i + 1)))
    return np.asarray(idx, dtype=np.int64)


PERM = _perm()


def _analyze(mask_perm):
    """Compile-time plan from the (permuted) boolean mask.

    Returns (plan, mpack):
      plan: per query-chunk, tuple of items
            (kt, kw, kwv, c0, bbox-or-None, moff) where
              kwv  = even-padded count of visible key rows computed,
              c0   = first (even-aligned) visible query column,
              bbox = (br0, br1, bc0, bc1) of the masked (zero) region inside
                     the computed window [kwv rows x (n-c0) cols]; bc* are
                     relative to c0; moff = column offset into mpack.
      mpack: [128, Wtot] float packed mask bounding boxes (S^T layout).
    """
    plan = []
    cols = []
    widths = 0
    cioff = []
    for (q0, q1) in QCH:
        ci_start = widths
        sub = mask_perm[q0:q1, :]
        nq = q1 - q0
        items = []
        for kt in range(NKT):
            k0, k1 = kt * KT, min((kt + 1) * KT, T)
            mt = sub[:, k0:k1].T          # [kw, nq]  S^T layout
            if not mt.any():
                continue
            kw = k1 - k0
            cvis = np.nonzero(mt.any(axis=0))[0]
            c0 = int(cvis[0]) & ~1
            c1 = int(cvis[-1]) + 1
            c1 += c1 & 1
            # c1m == -1 means "extends to the (padded) chunk end"
            c1m = -1 if c1 >= nq else c1
            rvis = np.nonzero(mt.any(axis=1))[0]
            r1 = int(rvis[-1]) + 1
            kwv = r1 + (r1 & 1)
            # computed window: rows [0:kwv], cols [c0:c1]; rows >= kw (tile
            # pad) get False here but their V rows are zeroed, so either way
            # they contribute nothing.
            cw = min(c1, nq)
            win = np.zeros((kwv, cw - c0), dtype=bool)
            rlim = min(r1, kw)
            win[0:rlim, :] = mt[0:rlim, c0:cw]
            if kwv > kw:
                win[kw:kwv, :] = True     # pad rows: harmless (V rows zero)
            z = ~win
            if z.any():
                rr = np.nonzero(z.any(axis=1))[0]
                cc = np.nonzero(z.any(axis=0))[0]
                br0, br1 = int(rr[0]), int(rr[-1]) + 1
                br0 = 0 if br0 < 64 else 64      # vector-engine partition window
                bc0, bc1 = int(cc[0]), int(cc[-1]) + 1
                frag = np.ones((128, bc1 - bc0), np.float32)
                frag[br0:br1, :] = win[br0:br1, bc0:bc1].astype(np.float32)
                items.append((kt, kw, kwv, c0, c1m, (br0, br1, bc0, bc1), widths))
                cols.append(frag)
                widths += bc1 - bc0
            else:
                items.append((kt, kw, kwv, c0, c1m, None, 0))
        # reorder: first item must cover [0:n] (start) and last item must
        # cover [0:n] (stop); move the second full-width item to the end.
        full = [i for i, it in enumerate(items) if it[3] == 0 and it[4] == -1]
        assert len(full) >= 2 and full[0] == 0, "need >=2 full-width items"
        j = full[1]
        items = [items[0]] + items[1:j] + items[j + 1:] + [items[j]]
        plan.append(tuple(items))
        cioff.append((ci_start, widths))
    if widths == 0:
        mpack = np.zeros((128, 4), np.float32)
    else:
        mpack = np.concatenate(cols, axis=1)
    return tuple(plan), np.ascontiguousarray(mpack), tuple(cioff)


_BUILD_CACHE = {}


def _split_excess_waits(nc, max_waits=1):
    """walrus (this build) rejects instructions with >2 sem-wait commands.

    Tile's kernel-tail drain waits on every live semaphore in one Drain;
    split the excess onto preceding same-engine instructions (extra Drains
    for InstDrain, NoOps otherwise).
    """
    import copy

    for bb in nc.main_func.blocks:
        insts = bb.instructions
        i = 0
        while i < len(insts):
            ins = insts[i]
            si = ins.sync_info
            mw = max_waits
            if si is not None and len(si.on_wait) > mw:
                waits = list(si.on_wait)
                extra = waits[:-mw]
                newones = []
                for j in range(0, len(extra), max_waits):
                    if ins.__class__.__name__ == "InstDrain":
                        d = mybir.InstDrain(
                            name=f"{ins.name}-sw{j}", engine=ins.engine
                        )
                    else:
                        d = mybir.InstNoOp(name=f"{ins.name}-sw{j}", engine=ins.engine)
                    si2 = copy.deepcopy(si)
                    si2.on_wait = extra[j:j + max_waits]
                    si2.on_update = []
                    d.sync_info = si2
                    newones.append(d)
                si.on_wait = waits[-mw:]
                for d in reversed(newones):
                    insts.insert(i, d)
                i += len(newones)
            i += 1


def _build(plan, wtot, cioff, split=True):
    key = (tuple(plan), wtot, cioff, split)
    if key in _BUILD_CACHE:
        return _BUILD_CACHE[key]

    nc = bass.Bass()
    xT = nc.declare_dram_parameter("xT", [C, TP], F16, isOutput=False)
    wa = nc.declare_dram_parameter("wa", [C, 3 * NHG * HD], F16, isOutput=False)
    wp = nc.declare_dram_parameter("wp", [NHG * HD, C], F16, isOutput=False)
    mp = nc.declare_dram_parameter("mp", [128, max(wtot, 4)], F16, isOutput=False)
    out = nc.declare_dram_parameter("out", [T, C], F32, isOutput=True)

    with tile.TileContext(nc) as tc:
        with ExitStack() as ctx:
            const = ctx.enter_context(tc.tile_pool(name="const", bufs=1))

            # ---- SBUF-resident tensors -------------------------------
            xt_sb = [const.tile([128, TP], F16, tag=f"xt{k}", name=f"xt{k}")
                     for k in range(KC)]
            wa_sb = [const.tile([128, 3 * NHG * HD], F16, tag=f"wa{k}", name=f"wa{k}")
                     for k in range(KC)]
            wp_sb = [const.tile([128, C], F16, tag=f"wp{k}", name=f"wpt{k}")
                     for k in range(3)]
            qt_sb = [const.tile([128, TP], F16, tag=f"qt{p}", name=f"qt{p}")
                     for p in range(NPACK)]
            kt_sb = [const.tile([128, TP], F16, tag=f"kt{p}", name=f"ktt{p}")
                     for p in range(NPACK)]
            v6_sb = [const.tile([128, VW], F16, tag=f"v6{t}", name=f"v6{t}")
                     for t in range(NKT)]
            yt_sb = [const.tile([128, TP], F16, tag=f"yt{p}", name=f"yt{p}")
                     for p in range(NPACK)]
            mkall = const.tile([128, max(wtot, 4)], F16, tag="mkall", name="mkall")

            # warm the scalar engine's Exp/Ln activation table during the
            # input-DMA window: the lazy ACT_TABLE_LOAD (~1.3us) otherwise
            # lands on the critical path at the first real exp.
            warm = const.tile([2, 16], F32, tag="warm", name="warm")
            nc.vector.memset(warm[:, :], 1.0)
            nc.scalar.activation(warm[0:2, 8:16], warm[0:2, 0:8], AF.Exp)


            # ---- DMA issue order -------------------------------------
            # x/wa interleaved per contraction tile (first chain starts
            # ~1us in); mask-frag ranges woven in, ordered by the chunk
            # processing order [1, 2, 3, 0]; wp last (needed only by the
            # output projection, which starts much later).
            def dma_mk(ci):
                m0, m1 = cioff[ci]
                if m1 > m0:
                    nc.sync.dma_start(out=mkall[:, m0:m1], in_=mp[:, m0:m1])

            for k in range(KC):
                nc.sync.dma_start(out=xt_sb[k][:, :], in_=xT[k * 128:(k + 1) * 128, :])
                nc.sync.dma_start(out=wa_sb[k][:, :], in_=wa[k * 128:(k + 1) * 128, :])
            for ci in (1, 2, 3, 0):
                dma_mk(ci)
            for k in range(3):
                nc.sync.dma_start(out=wp_sb[k][:, :], in_=wp[k * 128:(k + 1) * 128, :])

            # ---- emission helpers ------------------------------------
            def qkv_chain(pool, p, ci, j):
                """Q^T (j=0) or K^T (j=1) chain for pack p, chunk ci."""
                q0, q1 = QCHC[ci]
                n = q1 - q0
                dst = (qt_sb, kt_sb)[j][p]
                ps = pool.tile([128, 512], F32, tag="pj", name="pjt")
                col = j * NHG * HD + p * 128
                for k in range(KC):
                    nc.tensor.matmul(
                        ps[:, 0:n],
                        wa_sb[k][:, col:col + 128],
                        xt_sb[k][:, q0:q1],
                        start=(k == 0), stop=(k == KC - 1),
                    )
                nc.vector.tensor_copy(dst[:, q0:q1], ps[:, 0:n])

            def v_chain(pool, t):
                tw = min(128, T - t * KT)
                twp = tw + (tw & 1)
                ps = pool.tile([128, 512], F32, tag="pj", name="pjt")
                for k in range(KC):
                    nc.tensor.matmul(
                        ps[0:twp, 0:NHG * HD],
                        xt_sb[k][:, t * KT:t * KT + twp],
                        wa_sb[k][:, 2 * NHG * HD:3 * NHG * HD],
                        start=(k == 0), stop=(k == KC - 1),
                    )
                v6v = v6_sb[t].rearrange("a (h d) -> a h d", d=HD1)
                if tw < 128:
                    nc.gpsimd.memset(v6_sb[t][:, :], 0.0)
                psv = ps[:, 0:NHG * HD].rearrange("a (h d) -> a h d", d=HD)
                nc.vector.tensor_copy(v6v[0:tw, :, 0:HD], psv[0:tw, :, :])
                nc.gpsimd.memset(v6v[0:tw, :, HD:HD1], 1.0)

            def d_chain(pool, osb, t, half, wide=False):
                tw = min(128, T - t * KT)
                twp = tw + (tw & 1)
                n0, n1 = (0, 384) if half == 0 else (384, 768)
                if wide:
                    # borrow an idle sps slot, using its native tag/shape
                    po = pool.tile([128, 2, 512], F32, tag="s", name="st")[:, 0, :]
                else:
                    po = pool.tile([128, 512], F32, tag="pj", name="pjt")
                for k3 in range(3):
                    nc.tensor.matmul(
                        po[0:twp, 0:384],
                        yt_sb[k3][:, t * KT:t * KT + twp],
                        wp_sb[k3][:, n0:n1],
                        start=(k3 == 0), stop=(k3 == 2),
                    )
                ot = osb.tile([128, 384], F32, tag="ot", name="ot_sb")
                nc.vector.tensor_copy(ot[0:tw, :], po[0:tw, :384])
                nc.sync.dma_start(
                    out=out[t * KT:t * KT + tw, n0:n1], in_=ot[0:tw, :]
                )

            # ---- stage 1: chains needed before attention -------------
            # chunks are processed in order [1, 2, 3, 0]: chunk 0 holds the
            # action queries (which see every key tile), so it runs last,
            # after all kt/v6 tiles exist.  C(1) needs kt tiles 0..6 (in
            # key-chunks 0..2) and v6 tiles 0..6.
            #
            # Wave A interleaves 8 accumulation chains c-tile-major so the
            # PE always has a ready matmul while the x/wa DMAs stream in
            # (chain-major order would block in-order PE issue on the last
            # c-tile's DMA).  Wave B runs after the DMAs have landed.
            def chain_mm(ps, spec, k, start, stop):
                kind, a, b_ = spec
                if kind == "v":
                    t = a
                    tw = min(128, T - t * KT)
                    twp = tw + (tw & 1)
                    nc.tensor.matmul(
                        ps[0:twp, 0:NHG * HD],
                        xt_sb[k][:, t * KT:t * KT + twp],
                        wa_sb[k][:, 2 * NHG * HD:3 * NHG * HD],
                        start=start, stop=stop,
                    )
                else:
                    p, ci, j = a, b_, 0 if kind == "q" else 1
                    q0, q1 = QCHC[ci]
                    col = j * NHG * HD + p * 128
                    nc.tensor.matmul(
                        ps[:, 0:q1 - q0],
                        wa_sb[k][:, col:col + 128],
                        xt_sb[k][:, q0:q1],
                        start=start, stop=stop,
                    )

            def chain_fin(ps, spec):
                kind, a, b_ = spec
                if kind == "v":
                    t = a
                    tw = min(128, T - t * KT)
                    v6v = v6_sb[t].rearrange("a (h d) -> a h d", d=HD1)
                    if tw < 128:
                        # rows >= tw must be zero (pad keys contribute 0)
                        nc.gpsimd.memset(v6_sb[t][:, :], 0.0)
                    psv = ps[:, 0:NHG * HD].rearrange("a (h d) -> a h d", d=HD)
                    nc.vector.tensor_copy(v6v[0:tw, :, 0:HD], psv[0:tw, :, :])
                    nc.gpsimd.memset(v6v[0:tw, :, HD:HD1], 1.0)
                else:
                    p, ci = a, b_
                    q0, q1 = QCHC[ci]
                    dst = (qt_sb, kt_sb)[0 if kind == "q" else 1][p]
                    nc.vector.tensor_copy(dst[:, q0:q1], ps[:, 0:q1 - q0])

            # minimal prefix: exactly what C(1) pack 0's S matmuls need up
            # front (4 chains, c-tile-major so the PE tracks the DMA
            # stream).  Their DVE copies then overlap the v6 0..3 chains,
            # so C(1)'s first item starts ~2us earlier than with all 8
            # chains in one wave (whose 8 copies all queue at DMA-end).
            # Other packs' chains and v6 4..6 are fillers inside C(1).
            waveA = [("k", 0, 0), ("k", 0, 1), ("k", 0, 2), ("q", 0, 1)]
            with tc.tile_pool(name="pb", bufs=8, space="PSUM") as pb:
                psA = [pb.tile([128, 512], F32, tag="pj", name="pjt")
                       for _ in waveA]
                for k in range(KC):
                    for ps, spec in zip(psA, waveA):
                        chain_mm(ps, spec, k, k == 0, k == KC - 1)
                for ps, spec in zip(psA, waveA):
                    chain_fin(ps, spec)
                for t in range(4):
                    ps = pb.tile([128, 512], F32, tag="pj", name="pjt")
                    for k in range(KC):
                        chain_mm(ps, ("v", t, None), k, k == 0, k == KC - 1)
                    chain_fin(ps, ("v", t, None))

            # ---- main interleaved phase ------------------------------
            with tc.tile_pool(name="sps", bufs=2, space="PSUM") as sps, \
                 tc.tile_pool(name="ups", bufs=2, space="PSUM") as ups, \
                 tc.tile_pool(name="proj", bufs=2, space="PSUM") as proj, \
                 tc.tile_pool(name="epool", bufs=4) as epool, \
                 tc.tile_pool(name="npool", bufs=6) as npool, \
                 tc.tile_pool(name="osb", bufs=2) as osb:

                def emit_pv(p, u2, et, kt, kwv, c0, c1e, first, last):
                    for e in (0, 1):
                        h = 2 * p + e
                        nc.tensor.matmul(
                            u2[e][0:HD1, c0:c1e],
                            v6_sb[kt][0:kwv, h * HD1:(h + 1) * HD1],
                            et[0:kwv, e, c0:c1e],
                            start=first, stop=last,
                        )

                # filler emitters per chunk index, consumed between items.
                # chunk processing order is [1, 2, 3, 0]; a filler emitted
                # during chunk ci may only depend on chunks processed before.
                def fillers_for(ci):
                    fs = []
                    if ci == 1:
                        # v6 4..6 first: C(1) pack 0's own items read them a
                        # few slots after the weave emits them.
                        for t in (4, 5, 6):
                            fs.append(lambda t=t: v_chain(proj, t))
                        for p in (1, 2):         # pack p's qt/kt for C(1)
                            fs.append(lambda p=p: qkv_chain(proj, p, 1, 0))
                            fs.append(lambda p=p: qkv_chain(proj, p, 0, 1))
                            fs.append(lambda p=p: qkv_chain(proj, p, 1, 1))
                            fs.append(lambda p=p: qkv_chain(proj, p, 2, 1))
                        for t in (7, 8, 9, 10):  # C(2) needs v6 7..10
                            fs.append(lambda t=t: v_chain(proj, t))
                        for p in range(NPACK):   # C(2) needs kt ch3, qt ch2
                            fs.append(lambda p=p: qkv_chain(proj, p, 3, 1))
                        for p in range(NPACK):
                            fs.append(lambda p=p: qkv_chain(proj, p, 2, 0))
                    elif ci == 2:
                        for t in (11, 12, 13):
                            fs.append(lambda t=t: v_chain(proj, t))
                        for p in range(NPACK):
                            fs.append(lambda p=p: qkv_chain(proj, p, 3, 0))  # qt ch3
                        for t in (4, 5):         # rows in [512,768) < ch1 end
                            for h in (0, 1):
                                fs.append(lambda t=t, h=h: d_chain(proj, osb, t, h))
                    elif ci == 3:
                        for p in range(NPACK):
                            fs.append(lambda p=p: qkv_chain(proj, p, 0, 0))  # qt ch0
                        for t in (6, 7, 8):      # rows in [768,1152) < ch2 end
                            for h in (0, 1):
                                fs.append(lambda t=t, h=h: d_chain(proj, osb, t, h))
                    elif ci == 0:
                        for t in (9, 10, 11, 12, 13):   # rows >= 1152, ch2+ch3
                            for h in (0, 1):
                                fs.append(lambda t=t, h=h: d_chain(proj, osb, t, h))
                    return fs

                for ci in (1, 2, 3, 0):
                    q0, q1 = QCHC[ci]
                    n = q1 - q0
                    items = plan[ci]
                    fillers = fillers_for(ci)
                    fi = 0
                    # spread fillers evenly across this chunk's items
                    tot = len(items) * NPACK
                    done = 0
                    for p in range(NPACK):
                        u2 = [ups.tile([HD1, 448], F32, tag="u", name="ut")
                              for _ in (0, 1)]
                        pend = None   # deferred PV args
                        for ii, (kt, kw, kwv, c0, c1m, bbox, moff) in enumerate(items):
                            c1e = n if c1m < 0 else c1m
                            st = sps.tile([128, 2, 512], F32, tag="s", name="st")
                            et = epool.tile([128, 2, 512], F16, tag="e", name="et")
                            for e in (0, 1):
                                nc.tensor.matmul(
                                    st[0:kwv, e, c0:c1e],
                                    kt_sb[p][e * 64:(e + 1) * 64,
                                             kt * KT:kt * KT + kwv],
                                    qt_sb[p][e * 64:(e + 1) * 64,
                                             q0 + c0:q0 + c1e],
                                    start=True, stop=True,
                                )
                            nc.scalar.activation(
                                et[0:kwv, :, c0:c1e], st[0:kwv, :, c0:c1e],
                                AF.Exp, scale=0.125,
                            )
                            if bbox is not None:
                                br0, br1, bc0, bc1 = bbox
                                ets = et[br0:br1, :, c0 + bc0:c0 + bc1]
                                nc.vector.tensor_mul(
                                    ets,
                                    ets,
                                    mkall[br0:br1, moff:moff + bc1 - bc0]
                                    .rearrange("a (o w) -> a o w", o=1)
                                    .to_broadcast(ets.shape),
                                )
                            if pend is not None:
                                emit_pv(p, u2, *pend)
                            pend = (et, kt, kwv, c0, c1e, ii == 0, False)
                            done += 1
                            while fi * tot < done * len(fillers):
                                fillers[fi]()
                                fi += 1
                        et, kt, kwv, c0, c1e, first, _ = pend
                        emit_pv(p, u2, et, kt, kwv, c0, c1e, first, True)
                        # softmax normalization: PV already replicated the
                        # rowsum into u2 partitions 64..127 (ones-cols in V),
                        # so this is a pure-DVE reciprocal + multiply.
                        # 1/rowsum as Exp(-Ln(r)): stays inside the Exp/Ln
                        # activation table (a Reciprocal activation would
                        # trigger a ~1.3us ACT_TABLE_LOAD on every switch).
                        for e in (0, 1):
                            lr = npool.tile([64, 448], F32, tag="lr", name="lr")
                            nc.scalar.activation(
                                lr[:, 0:n], u2[e][64:128, 0:n], AF.Ln
                            )
                            rs = npool.tile([64, 448], F32, tag="rs", name="rs")
                            nc.scalar.activation(
                                rs[:, 0:n], lr[:, 0:n], AF.Exp, scale=-1.0
                            )
                            nc.vector.tensor_mul(
                                yt_sb[p][e * 64:(e + 1) * 64, q0:q1],
                                u2[e][0:64, 0:n],
                                rs[:, 0:n],
                            )
                    while fi < len(fillers):
                        fillers[fi]()
                        fi += 1

                # ---- tail: output tiles touching chunk-0 rows --------
                # borrow the now-idle sps slots so 4 chains pipeline
                for i, (t, h) in enumerate(
                        [(t, h) for t in (0, 1, 2, 3) for h in (0, 1)]):
                    if i % 2 == 0:
                        d_chain(proj, osb, t, h)
                    else:
                        d_chain(sps, osb, t, h, wide=True)

    if split:
        _split_excess_waits(nc)
    _BUILD_CACHE[key] = nc
    return nc


def _prep_inputs(x, W_attn, W_proj, mpack):
    """Per-core input maps. core c -> batch c//2, head-group c%2."""
    x = np.asarray(x, np.float32)
    W_attn = np.asarray(W_attn, np.float32)
    W_proj = np.asarray(W_proj, np.float32)
    mpack16 = mpack.astype(np.float16)
    in_maps = []
    xT_by_batch = []
    for b in range(B):
        xt = np.zeros((C, TP), np.float16)
        xt[:, :T] = x[b][PERM, :].T.astype(np.float16)
        xT_by_batch.append(xt)
    for c in range(NCORES):
        b, g = c // 2, c % 2
        cs = slice(g * NHG * HD, (g + 1) * NHG * HD)
        wa_s = np.ascontiguousarray(
            np.concatenate(
                [W_attn[:, cs], W_attn[:, C:][:, cs], W_attn[:, 2 * C:][:, cs]],
                axis=1,
            ).astype(np.float16)
        )
        wp_s = np.ascontiguousarray(W_proj[cs, :].astype(np.float16))
        in_maps.append(
            {"xT": xT_by_batch[b], "wa": wa_s, "wp": wp_s, "mp": mpack16}
        )
    return in_maps


def _run(inputs, trace=False, trace_cores=None):
    x = np.asarray(inputs["x"], np.float32)
    mask = np.asarray(inputs["mask"], bool)
    mask_perm = mask[np.ix_(PERM, PERM)]
    plan, mpack, cioff = _analyze(mask_perm)
    nc = _build(plan, mpack.shape[1], cioff)
    in_maps = _prep_inputs(x, inputs["W_attn"], inputs["W_proj"], mpack)
    res = run_bass_kernel_spmd(
        nc, in_maps, list(range(NCORES)), trace=trace, trace_cores=trace_cores
    )
    outs = [np.asarray(r["out"], np.float32) for r in res.results]
    full = np.empty((B, T, C), np.float32)
    for b in range(B):
        comb = outs[2 * b] + outs[2 * b + 1]
        full[b][PERM, :] = comb
    return full, res


def kernel(**inputs) -> np.ndarray:
    out, _ = _run(inputs)
    return out
